# revision 1
# baseline (speedup 1.0000x reference)
import os
import sys

sys.path.insert(0, "/opt/trn_rl_repo")

import numpy as np
import ml_dtypes

import concourse.bass as bass
import concourse.bacc as bacc
import concourse.mybir as mybir
from concourse import masks
from concourse.bass_utils import run_bass_kernel_spmd
from concourse.tile import TileContext

S = 1024
DIM = 2560
HD = 128
NH = 20
NKV = 5
GS = 128
THETA = 500000.0
EPS = 1e-05
KBASE = NH * HD            # k rows start in w_qkv
VBASE = KBASE + NKV * HD   # v rows start
NC = 8
KCH = DIM // 128           # 20 k-chunks
WQCOLS = 7 * 128           # [qs0 qs1 qs2 kA vA kB vB]
OC = DIM // NC             # 320 output cols per core
MT = S // 128              # 8 token tiles

# head assignment per core: [slot0, slot1, slot2]; None = garbage slot
HEADS = [
    [0, 1, 8], [2, 3, 9], [4, 5, 10], [6, 7, 11],
    [12, 13, None], [14, 15, None], [16, 17, None], [18, 19, None],
]
GA = [0, 0, 1, 1, 3, 3, 4, 4]              # kv group for slots 0,1
GB = [2, 2, 2, 2, None, None, None, None]  # kv group for slot 2
REAL_CHUNKS = [j * 3 + s for j in range(NC) for s in range(3) if HEADS[j][s] is not None]
assert len(REAL_CHUNKS) == NH

FP16 = np.float16
SCALE = float(HD) ** -0.5
ESHIFT = -2.0  # exp(score*SCALE + ESHIFT); cancels in softmax ratio

_cached = {}


def _build_nc():
    nc = bacc.Bacc("TRN2", target_bir_lowering=False, debug=False, num_devices=NC)
    f32 = mybir.dt.float32
    f16 = mybir.dt.float16
    i16 = mybir.dt.int16

    x_d = nc.declare_dram_parameter("x", [S, DIM], f32, isOutput=False)
    wq_d = nc.declare_dram_parameter("wq", [DIM, WQCOLS], f16, isOutput=False)
    wo_d = nc.declare_dram_parameter("wo", [NC * 384, OC], f16, isOutput=False)
    tq1_d = nc.declare_dram_parameter("tq1", [S, HD], f32, isOutput=False)
    tq2_d = nc.declare_dram_parameter("tq2", [S, HD], f32, isOutput=False)
    tk1_d = nc.declare_dram_parameter("tk1", [S, HD], f32, isOutput=False)
    tk2_d = nc.declare_dram_parameter("tk2", [S, HD], f32, isOutput=False)
    # 4 causal mask variants for 512-wide score groups: r = kc - 4*grp
    cmask_d = nc.declare_dram_parameter("cmask", [4 * 128, 512], f16, isOutput=False)
    out_d = nc.declare_dram_parameter("out", [S, OC], f32, isOutput=True)

    q16_d = nc.dram_tensor("q16d", [S, DIM], i16, kind="Internal")
    agin_a = nc.dram_tensor("agina", [384, S // 2], f16, kind="Internal")
    agin_b = nc.dram_tensor("aginb", [384, S // 2], f16, kind="Internal")
    agout_a = nc.dram_tensor("agouta", [NC * 384, S // 2], f16, kind="Internal",
                             addr_space="Shared")
    agout_b = nc.dram_tensor("agoutb", [NC * 384, S // 2], f16, kind="Internal",
                             addr_space="Shared")
    lmin_d = [nc.dram_tensor(f"lmin{h}", [1, S // 2], f32, kind="Internal")
              for h in range(2)]
    lmout_d = [nc.dram_tensor(f"lmout{h}", [NC, S // 2], f32, kind="Internal",
                              addr_space="Shared") for h in range(2)]
    ff_d = nc.dram_tensor("ffd", [4, S], f32, kind="Internal")
    qrope_d = nc.dram_tensor("qroped", [5 * S, HD], f16, kind="Internal")

    with TileContext(nc) as tc:
        with (
            tc.tile_pool(name="cst", bufs=1) as cst,
            tc.tile_pool(name="kvsb", bufs=1) as kvsb,
            tc.tile_pool(name="arawp", bufs=1) as arawp,
            tc.tile_pool(name="rows", bufs=1) as rows,
            tc.tile_pool(name="nrp", bufs=2) as nrp,
        ):
            ident_h = cst.tile([128, 128], f16, tag="idh", name="idh")
            masks.make_identity(nc, ident_h[:, :])
            ident_f = cst.tile([128, 128], f32, tag="idf", name="idf")
            masks.make_identity(nc, ident_f[:, :])
            ones_row = cst.tile([1, 128], f32, tag="ones", name="ones")
            nc.vector.memset(ones_row[:, :], 1.0)
            eshift = cst.tile([128, 1], f32, tag="esh", name="esh")
            nc.vector.memset(eshift[:, :], ESHIFT)
            cmask = cst.tile([128, 4, 512], f16, tag="cm", name="cm")
            nc.sync.dma_start(out=cmask[:, :, :],
                              in_=cmask_d.ap().rearrange("(r p) n -> p r n", p=128))

            tabs = {}
            for nm, d in (("tq1", tq1_d), ("tq2", tq2_d),
                          ("tk1", tk1_d), ("tk2", tk2_d)):
                t = cst.tile([128, MT, HD], f32, tag=f"tb{nm}", name=f"tb{nm}")
                nc.sync.dma_start(out=t[:, :, :],
                                  in_=d.ap().rearrange("(m p) d -> p m d", p=128))
                for m in range(MT):
                    tabs[(nm, m)] = t[:, m, :]

            s_cols = cst.tile([128, MT], f32, tag="scols", name="scols")
            rs_cols = cst.tile([128, MT], f32, tag="rscols", name="rscols")

            KT = [kvsb.tile([128, S], f16, tag=f"KT{b}", name=f"KT{b}") for b in range(2)]
            VV = [[kvsb.tile([128, 128], f16, tag=f"V{b}_{m}", name=f"V{b}_{m}")
                   for m in range(MT)] for b in range(2)]
            qT = [kvsb.tile([128, S], f16, tag=f"qT{s}", name=f"qT{s}") for s in range(3)]
            araw = [arawp.tile([128, S], f32, tag=f"araw{s}", name=f"araw{s}")
                    for s in range(3)]
            rden_cols = [rows.tile([128, MT], f32, tag=f"rdc{s}", name=f"rdc{s}")
                         for s in range(3)]
            lm_cols = rows.tile([128, MT], f32, tag="lmc", name="lmc")

            # rope output staging: [p, 5 slots, MT, d] fp16 (q0,q1,q2,kA,kB)
            rbp = tc.tile_pool(name="rbp", bufs=1)
            rbq = rbp.__enter__().tile([128, 5, MT, HD], f16, tag="rbq", name="rbq")

            def norm_rope_batched(eng, xn_view, t1, t2, ob_view, scratch_tag):
                """xn_view [128, nh, 128] normalized input; tables [128, nh*128];
                writes roped fp16 into ob_view [128, nh, 128]."""
                nh = xn_view.shape[1]
                se = xn_view.rearrange("p h (i two) -> p h i two", two=2)
                t1b = t1.rearrange("p (one d) -> p one d", one=1).to_broadcast(
                    [128, nh, HD])
                t2b = t2.rearrange("p (one d) -> p one d", one=1).to_broadcast(
                    [128, nh, HD])
                t1e = t1b.rearrange("p h (i two) -> p h i two", two=2)
                t2e = t2b.rearrange("p h (i two) -> p h i two", two=2)
                ob = ob_view.rearrange("p h (i two) -> p h i two", two=2)
                a1 = nrp.tile([128, nh, 64], f32, tag=f"ra1{scratch_tag}",
                              name=f"ra1{scratch_tag}")
                a2 = nrp.tile([128, nh, 64], f32, tag=f"ra2{scratch_tag}",
                              name=f"ra2{scratch_tag}")
                eng.tensor_mul(a1[:, :, :], se[:, :, :, 0], t1e[:, :, :, 0])
                eng.tensor_mul(a2[:, :, :], se[:, :, :, 1], t2e[:, :, :, 1])
                eng.tensor_sub(ob[:, :, :, 0], a1[:, :, :], a2[:, :, :])
                eng.tensor_mul(a1[:, :, :], se[:, :, :, 0], t2e[:, :, :, 0])
                eng.tensor_mul(a2[:, :, :], se[:, :, :, 1], t1e[:, :, :, 1])
                eng.tensor_add(ob[:, :, :, 1], a1[:, :, :], a2[:, :, :])

            with (
                tc.tile_pool(name="wqp", bufs=KCH) as wqp,
                tc.tile_pool(name="qtp", bufs=KCH) as qtp,
            ):
                wq_sb = []
                for kc in range(KCH):
                    t = wqp.tile([128, WQCOLS], f16, tag="wq", name="wq")
                    nc.scalar.dma_start(out=t[:, :], in_=wq_d[kc * 128:(kc + 1) * 128, :])
                    wq_sb.append(t)

                q8T = [qtp.tile([128, S], f16, tag="q8T", name="q8T", uniquify=True)
                       for _ in range(KCH)]

                # ---- Stages A+B interleaved by token halves ----
                with (
                    tc.tile_pool(name="xa", bufs=2) as xap,
                    tc.tile_pool(name="q16", bufs=3) as q16p,
                    tc.tile_pool(name="qti", bufs=4) as qtip,
                ):
                    for half in range(2):
                        for m in range(half * 4, half * 4 + 4):
                            xa = xap.tile([128, DIM], f32, tag="x", name="x")
                            nc.sync.dma_start(out=xa[:, :],
                                              in_=x_d[m * 128:(m + 1) * 128, :])
                            mx = xap.tile([128, 1], f32, tag="mx", name="mx")
                            nc.vector.tensor_reduce(mx[:, :], xa[:, :],
                                                    mybir.AxisListType.X,
                                                    mybir.AluOpType.max,
                                                    apply_absolute_value=True)
                            mx2 = xap.tile([128, 1], f32, tag="mx2", name="mx2")
                            nc.vector.tensor_scalar_max(mx2[:, :], mx[:, :], 1e-5)
                            rmx = xap.tile([128, 1], f32, tag="rmx", name="rmx")
                            nc.vector.reciprocal(rmx[:, :], mx2[:, :])
                            nc.vector.tensor_scalar_mul(s_cols[:, m:m + 1], rmx[:, :],
                                                        127.0)
                            nc.vector.tensor_scalar_mul(rs_cols[:, m:m + 1], mx2[:, :],
                                                        1.0 / 127.0)
                            q16 = q16p.tile([128, DIM], i16, tag="q16", name="q16")
                            nc.scalar.activation(q16[:, :], xa[:, :],
                                                 mybir.ActivationFunctionType.Copy,
                                                 scale=s_cols[:, m:m + 1])
                            nc.sync.dma_start(out=q16_d[m * 128:(m + 1) * 128, :],
                                              in_=q16[:, :])
                        hs = slice(half * 512, half * 512 + 512)
                        for kc in range(KCH):
                            qti = qtip.tile([128, 512], i16, tag="qti", name="qti")
                            nc.sync.dma_start(
                                out=qti[:, :],
                                in_=q16_d[hs, kc * 128:(kc + 1) * 128],
                                transpose=True)
                            nc.vector.tensor_copy(q8T[kc][:, hs], qti[:, :])

                # ---- Stage C: qkv matmul + epilogues ----
                with (
                    tc.tile_pool(name="psq", bufs=3, space="PSUM") as psq,
                    tc.tile_pool(name="pstr", bufs=2, space="PSUM") as pstr,
                ):
                    for m in range(MT):
                        psA = psq.tile([128, 384], f32, tag="psA", name="psA")
                        psB = psq.tile([128, 512], f32, tag="psB", name="psB")
                        for kc in range(KCH):
                            lh = q8T[kc][:, m * 128:(m + 1) * 128]
                            nc.tensor.matmul(psA[:, :], lh, wq_sb[kc][:, 0:384],
                                             start=(kc == 0), stop=(kc == KCH - 1))
                            nc.tensor.matmul(psB[:, :], lh, wq_sb[kc][:, 384:896],
                                             start=(kc == 0), stop=(kc == KCH - 1))
                        rs_ap = rs_cols[:, m:m + 1]
                        # evacuate q (3 heads) and k (2 heads) f32; V scaled fp16
                        qxs = nrp.tile([128, 384], f32, tag="qxs", name="qxs")
                        nc.scalar.copy(qxs[:, :], psA[:, :])
                        kxs = nrp.tile([128, 2, 128], f32, tag="kxs", name="kxs")
                        nc.scalar.copy(kxs[:, :, :],
                                       psB.rearrange("p (b c) -> p b c", c=256)[:, :, 0:128])
                        for blk in range(2):
                            nc.scalar.activation(VV[blk][m][:, :],
                                                 psB[:, blk * 256 + 128:blk * 256 + 256],
                                                 mybir.ActivationFunctionType.Copy,
                                                 scale=rs_ap)
                        # rms factors for all 5 heads in one [128, 5] tile
                        sq = nrp.tile([128, 384], f32, tag="sqq", name="sqq")
                        sk = nrp.tile([128, 256], f32, tag="sqk", name="sqk")
                        nc.vector.tensor_mul(sq[:, :], qxs[:, :], qxs[:, :])
                        nc.vector.tensor_mul(sk[:, :], kxs.rearrange("p b c -> p (b c)"),
                                               kxs.rearrange("p b c -> p (b c)"))
                        rs5 = nrp.tile([128, 5], f32, tag="rs5", name="rs5")
                        nc.vector.tensor_reduce(rs5[:, 0:3],
                                                sq.rearrange("p (h d) -> p h d", d=128),
                                                mybir.AxisListType.X, mybir.AluOpType.add)
                        nc.vector.tensor_reduce(rs5[:, 3:5],
                                                sk.rearrange("p (h d) -> p h d", d=128),
                                                mybir.AxisListType.X, mybir.AluOpType.add)
                        nc.vector.tensor_scalar(rs5[:, :], rs5[:, :], 1.0 / HD, EPS,
                                                mybir.AluOpType.mult, mybir.AluOpType.add)
                        nc.vector.reciprocal(rs5[:, :], rs5[:, :])
                        nc.scalar.activation(rs5[:, :], rs5[:, :],
                                             mybir.ActivationFunctionType.Sqrt)
                        # normalize (per-head per-partition scalar) then rope
                        for h in range(3):
                            nc.vector.tensor_scalar_mul(qxs[:, h * 128:(h + 1) * 128],
                                                        qxs[:, h * 128:(h + 1) * 128],
                                                        rs5[:, h:h + 1])
                        for h in range(2):
                            nc.vector.tensor_scalar_mul(kxs[:, h, :], kxs[:, h, :],
                                                        rs5[:, 3 + h:4 + h])
                        norm_rope_batched(nc.vector,
                                          qxs.rearrange("p (h d) -> p h d", d=128),
                                          tabs[("tq1", m)], tabs[("tq2", m)],
                                          rbq[:, 0:3, m, :], "q")
                        norm_rope_batched(nc.gpsimd, kxs[:, :, :],
                                          tabs[("tk1", m)], tabs[("tk2", m)],
                                          rbq[:, 3:5, m, :], "k")

            # roped q/k -> DRAM -> transposed reload into qT/KT
            with nc.allow_non_contiguous_dma(reason="rope stage 256B rows"):
                for i in range(5):
                    nc.sync.dma_start(
                        out=qrope_d[i * S:(i + 1) * S, :]
                        .rearrange("(m p) d -> p m d", p=128),
                        in_=rbq[:, i, :, :])
            for i in range(5):
                dst = qT[i] if i < 3 else KT[i - 3]
                nc.sync.dma_start(out=dst[:, :], in_=qrope_d[i * S:(i + 1) * S, :],
                                  transpose=True)
            rbp.__exit__(None, None, None)

            # ---- Stage F: attention, 512-wide q groups, scoresT [k, q] ----
            with (
                tc.tile_pool(name="pssc", bufs=3, space="PSUM") as pssc,
                tc.tile_pool(name="psav", bufs=2, space="PSUM") as psav,
                tc.tile_pool(name="pstr2", bufs=1, space="PSUM") as pstr2,
                tc.tile_pool(name="ptt", bufs=12) as ptt,
                tc.tile_pool(name="accp", bufs=2) as accp,
            ):
                for grp in range(2):
                    for sl in range(3):
                        blk = 0 if sl < 2 else 1
                        gs = slice(grp * 512, grp * 512 + 512)
                        nkc = 4 * grp + 4
                        pts = []
                        acc = accp.tile([128, 512], f32, tag="acc", name="acc")
                        for kc in range(nkc):
                            ps = pssc.tile([128, 512], f32, tag="sc", name="sc")
                            nc.tensor.matmul(ps[:, :],
                                             KT[blk][:, kc * 128:(kc + 1) * 128],
                                             qT[sl][:, gs], start=True, stop=True)
                            pt = ptt.tile([128, 512], f16, tag="pt", name="pt")
                            nc.scalar.activation(pt[:, :], ps[:, :],
                                                 mybir.ActivationFunctionType.Exp,
                                                 bias=eshift[:, 0:1], scale=SCALE)
                            r = kc - 4 * grp
                            if r >= 0:
                                nc.vector.tensor_mul(pt[:, :], pt[:, :], cmask[:, r, :])
                            pts.append(pt)
                            if kc == 0:
                                nc.vector.tensor_copy(acc[:, :], pt[:, :])
                            else:
                                nc.vector.tensor_add(acc[:, :], acc[:, :], pt[:, :])
                        avp = psav.tile([128, 512], f32, tag="av", name="av")
                        for kc in range(nkc):
                            nc.tensor.matmul(avp[:, :], VV[blk][kc][:, :], pts[kc][:, :],
                                             start=(kc == 0), stop=(kc == nkc - 1))
                        nc.scalar.copy(araw[sl][:, gs], avp[:, :])
                        bsl = slice(grp * 4, grp * 4 + 4)
                        accT = pstr2.tile([128, 512], f32, tag="accT", name="accT")
                        at2 = pstr2.tile([128, 512], f32, tag="at2", name="at2")
                        for j in range(4):
                            nc.tensor.transpose(accT[:, j * 128:(j + 1) * 128],
                                                acc[:, j * 128:(j + 1) * 128],
                                                ident_f[:, :])
                            nc.tensor.transpose(at2[:, j * 128:(j + 1) * 128],
                                                araw[sl][:, (4 * grp + j) * 128:
                                                         (4 * grp + j + 1) * 128],
                                                ident_f[:, :])
                        rden4 = accp.tile([128, 4], f32, tag="rden4", name="rden4")
                        nc.vector.tensor_reduce(
                            rden4[:, :], accT.rearrange("p (j d) -> p j d", d=128),
                            mybir.AxisListType.X, mybir.AluOpType.add)
                        nc.vector.reciprocal(rden4[:, :], rden4[:, :])
                        nc.vector.tensor_copy(rden_cols[sl][:, bsl], rden4[:, :])
                        lmax4 = accp.tile([128, 4], f32, tag="lmax4", name="lmax4")
                        nc.vector.tensor_reduce(
                            lmax4[:, :], at2.rearrange("p (j d) -> p j d", d=128),
                            mybir.AxisListType.X, mybir.AluOpType.max,
                            apply_absolute_value=True)
                        nc.vector.tensor_mul(lmax4[:, :], lmax4[:, :], rden4[:, :])
                        if sl == 0:
                            nc.vector.tensor_copy(lm_cols[:, bsl], lmax4[:, :])
                        else:
                            nc.vector.tensor_tensor(lm_cols[:, bsl], lm_cols[:, bsl],
                                                    lmax4[:, :], mybir.AluOpType.max)
                    # fire this half's max AllGather while the other group runs
                    with nc.allow_non_contiguous_dma(reason="2KB scale rows"):
                        nc.sync.dma_start(
                            out=lmin_d[grp][0, :].rearrange("(m p) -> p m", p=128),
                            in_=lm_cols[:, grp * 4:grp * 4 + 4])
                    nc.gpsimd.collective_compute(
                        "AllGather", mybir.AluOpType.bypass,
                        ins=[lmin_d[grp].ap().opt()], outs=[lmout_d[grp].ap().opt()],
                        replica_groups=[list(range(NC))],
                    )

                # rden pre-scaling first (independent of the max collective);
                # scale-row DMAs ride the Scalar HWDGE queue so the Sync queue
                # (FIFO) is not blocked behind collective-gated loads.
                for sl in range(3):
                    with nc.allow_non_contiguous_dma(reason="4KB scale rows"):
                        nc.scalar.dma_start(
                            out=ff_d[sl, :].rearrange("(m p) -> p m", p=128),
                            in_=rden_cols[sl][:, :])
                    ffrow = rows.tile([1, S], f32, tag="ffrow", name="ffrow")
                    nc.scalar.dma_start(out=ffrow[:, :], in_=ff_d[sl:sl + 1, :])
                    fac = arawp.tile([128, S], f32, tag="fac", name="fac")
                    for h in range(2):
                        bp = pstr2.tile([128, 512], f32, tag="bc", name="bc")
                        nc.tensor.matmul(bp[:, :], ones_row[:, :],
                                         ffrow[:, h * 512:(h + 1) * 512],
                                         start=True, stop=True)
                        nc.scalar.copy(fac[:, h * 512:(h + 1) * 512], bp[:, :])
                    nc.vector.tensor_mul(araw[sl][:, :], araw[sl][:, :], fac[:, :])

                rso_cols = rows.tile([128, MT], f32, tag="rsoc", name="rsoc")
                with tc.tile_pool(name="qga", bufs=2) as qga:
                    for hf in range(2):
                        hsl = slice(hf * 512, hf * 512 + 512)
                        bsl = slice(hf * 4, hf * 4 + 4)
                        gma = rows.tile([128, NC, 4], f32, tag="gma", name="gma")
                        with nc.allow_non_contiguous_dma(reason="2KB scale rows"):
                            nc.sync.dma_start(
                                out=gma[:, :, :],
                                in_=lmout_d[hf].ap().rearrange("c (m p) -> p c m",
                                                               p=128))
                        nc.vector.tensor_tensor(gma[:, 0:4, :], gma[:, 0:4, :],
                                                gma[:, 4:8, :], mybir.AluOpType.max)
                        nc.vector.tensor_tensor(gma[:, 0:2, :], gma[:, 0:2, :],
                                                gma[:, 2:4, :], mybir.AluOpType.max)
                        gm = rows.tile([128, 4], f32, tag="gm", name="gm")
                        nc.vector.tensor_tensor(gm[:, :], gma[:, 0, :], gma[:, 1, :],
                                                mybir.AluOpType.max)
                        gclip = rows.tile([128, 4], f32, tag="gclip", name="gclip")
                        nc.vector.tensor_scalar_max(gclip[:, :], gm[:, :], 1e-5)
                        nc.vector.tensor_scalar_mul(rso_cols[:, bsl], gclip[:, :],
                                                    1.0 / 127.0)
                        so4 = rows.tile([128, 4], f32, tag="so4", name="so4")
                        nc.vector.reciprocal(so4[:, :], gclip[:, :])
                        nc.vector.tensor_scalar_mul(so4[:, :], so4[:, :], 127.0)
                        with nc.allow_non_contiguous_dma(reason="2KB scale rows"):
                            nc.scalar.dma_start(
                                out=ff_d[3, hf * 512:hf * 512 + 512]
                                .rearrange("(m p) -> p m", p=128),
                                in_=so4[:, :])
                        sorow = rows.tile([1, 512], f32, tag="sorow", name="sorow")
                        nc.scalar.dma_start(
                            out=sorow[:, :],
                            in_=ff_d[3:4, hf * 512:hf * 512 + 512])
                        bp = pstr2.tile([128, 512], f32, tag="bc", name="bc")
                        nc.tensor.matmul(bp[:, :], ones_row[:, :], sorow[:, :],
                                         start=True, stop=True)
                        sofac = qga.tile([128, 512], f32, tag="sofac", name="sofac")
                        nc.scalar.copy(sofac[:, :], bp[:, :])
                        agin_h = agin_a if hf == 0 else agin_b
                        agout_h = agout_a if hf == 0 else agout_b
                        for sl in range(3):
                            q16a = qga.tile([128, 512], i16, tag="q16a", name="q16a")
                            nc.vector.tensor_mul(q16a[:, :], araw[sl][:, hsl],
                                                 sofac[:, :])
                            aq = qga.tile([128, 512], f16, tag="aq", name="aq")
                            nc.vector.tensor_copy(aq[:, :], q16a[:, :])
                            nc.sync.dma_start(
                                out=agin_h[sl * 128:(sl + 1) * 128, :], in_=aq[:, :])
                        nc.gpsimd.collective_compute(
                            "AllGather", mybir.AluOpType.bypass,
                            ins=[agin_h.ap().opt()], outs=[agout_h.ap().opt()],
                            replica_groups=[list(range(NC))],
                        )

            # ---- Stage H: o_proj (per token half, overlapping the two AllGathers) ----
            with (
                tc.tile_pool(name="pso", bufs=4, space="PSUM") as pso,
                tc.tile_pool(name="agtp", bufs=2 * NH) as agtp,
                tc.tile_pool(name="wop", bufs=NH) as wop,
                tc.tile_pool(name="ogp", bufs=2) as ogp,
            ):
                wo_sb = []
                for ck in REAL_CHUNKS:
                    w = wop.tile([128, OC], f16, tag="wo", name="wo")
                    nc.scalar.dma_start(out=w[:, :], in_=wo_d[ck * 128:(ck + 1) * 128, :])
                    wo_sb.append(w)
                for hf, agout_h in ((0, agout_a), (1, agout_b)):
                    agt = []
                    for ck in REAL_CHUNKS:
                        t = agtp.tile([128, 512], f16, tag="agt", name="agt")
                        nc.sync.dma_start(out=t[:, :],
                                          in_=agout_h[ck * 128:(ck + 1) * 128, :])
                        agt.append(t)
                    for j in range(4):
                        m = hf * 4 + j
                        ps = pso.tile([128, OC], f32, tag="po", name="po")
                        for i in range(NH):
                            nc.tensor.matmul(ps[:, :],
                                             agt[i][:, j * 128:(j + 1) * 128],
                                             wo_sb[i][:, :], start=(i == 0),
                                             stop=(i == NH - 1))
                        og = ogp.tile([128, OC], f32, tag="og", name="og")
                        nc.scalar.activation(og[:, :], ps[:, :],
                                             mybir.ActivationFunctionType.Copy,
                                             scale=rso_cols[:, m:m + 1])
                        nc.sync.dma_start(out=out_d[m * 128:(m + 1) * 128, :],
                                          in_=og[:, :])

    nc.compile()
    return nc


def _host_prep(x, w_qkv, ws_qkv, w_o, ws_o, q_norm_w, k_norm_w):
    w_dq = (w_qkv * np.repeat(ws_qkv, GS, axis=1)).astype(np.float32)
    wo_dq = (w_o * np.repeat(ws_o, GS, axis=1)).astype(np.float32)

    pos = np.arange(S, dtype=np.float32)
    inv_freq = (THETA ** (-np.arange(0, HD, 2, dtype=np.float32) / HD)).astype(np.float32)
    ang = pos[:, None] * inv_freq[None, :]
    ce = np.repeat(np.cos(ang).astype(np.float32), 2, axis=1)
    se = np.repeat(np.sin(ang).astype(np.float32), 2, axis=1)
    tq1 = (ce * q_norm_w[None, :]).astype(np.float32)
    tq2 = (se * q_norm_w[None, :]).astype(np.float32)
    tk1 = (ce * k_norm_w[None, :]).astype(np.float32)
    tk2 = (se * k_norm_w[None, :]).astype(np.float32)

    # mask variants: scoresT [k(128), 512 q]; group cols = 4 q-blocks; r = kc-4*grp
    cm = np.zeros((4, 128, 512), np.float32)
    tri = np.triu(np.ones((128, 128), np.float32))  # keep k <= q
    for r in range(4):
        for j in range(4):
            if j > r:
                cm[r, :, j * 128:(j + 1) * 128] = 1.0
            elif j == r:
                cm[r, :, j * 128:(j + 1) * 128] = tri
    cmask = cm.reshape(4 * 128, 512).astype(FP16)

    in_maps = []
    for c in range(NC):
        wq = np.zeros((DIM, WQCOLS), np.float32)
        for sl in range(3):
            h = HEADS[c][sl]
            if h is not None:
                wq[:, sl * 128:(sl + 1) * 128] = w_dq[h * HD:(h + 1) * HD, :].T
        ga = GA[c]
        wq[:, 384:512] = w_dq[KBASE + ga * HD:KBASE + (ga + 1) * HD, :].T
        wq[:, 512:640] = w_dq[VBASE + ga * HD:VBASE + (ga + 1) * HD, :].T
        gb = GB[c]
        if gb is not None:
            wq[:, 640:768] = w_dq[KBASE + gb * HD:KBASE + (gb + 1) * HD, :].T
            wq[:, 768:896] = w_dq[VBASE + gb * HD:VBASE + (gb + 1) * HD, :].T

        wo = np.zeros((NC * 384, OC), np.float32)
        for j in range(NC):
            for sl in range(3):
                h = HEADS[j][sl]
                if h is not None:
                    rws = slice((j * 3 + sl) * 128, (j * 3 + sl) * 128 + 128)
                    wo[rws, :] = wo_dq[c * OC:(c + 1) * OC, h * HD:(h + 1) * HD].T

        in_maps.append({
            "x": x.astype(np.float32),
            "wq": wq.astype(FP16),
            "wo": wo.astype(FP16),
            "tq1": tq1, "tq2": tq2, "tk1": tk1, "tk2": tk2,
            "cmask": cmask,
        })
    return in_maps


def kernel(x, w_qkv, ws_qkv, w_o, ws_o, q_norm_w, k_norm_w):
    x = np.asarray(x, np.float32)
    w_qkv = np.asarray(w_qkv, np.float32)
    ws_qkv = np.asarray(ws_qkv, np.float32)
    w_o = np.asarray(w_o, np.float32)
    ws_o = np.asarray(ws_o, np.float32)
    q_norm_w = np.asarray(q_norm_w, np.float32)
    k_norm_w = np.asarray(k_norm_w, np.float32)

    if "nc" not in _cached:
        _cached["nc"] = _build_nc()
    nc = _cached["nc"]

    in_maps = _host_prep(x, w_qkv, ws_qkv, w_o, ws_o, q_norm_w, k_norm_w)
    trace = bool(int(os.environ.get("BENCH_TRACE", "0")))
    res = run_bass_kernel_spmd(nc, in_maps, core_ids=list(range(NC)), trace=trace)
    if trace and res.exec_time_ns is not None:
        print(f"HW exec time: {res.exec_time_ns} ns")
        _cached["exec_time_ns"] = res.exec_time_ns

    out = np.concatenate([np.asarray(res.results[c]["out"], np.float32)
                          for c in range(NC)], axis=1)
    return out

